# revision 1
# baseline (speedup 1.0000x reference)
"""Causal multi-head attention on 8 trn2 NeuronCores.

Problem: B=4, S=2048, D=2048, H=16 heads, head_dim=128, causal softmax,
torch-style Linear projections (W stored [in, out]).

Sharding: core c handles batch b = c//2 and head-group g = c%2
(8 heads = 1024 output columns of Wq/Wk/Wv, 1024 rows of Wo).
Each core produces a partial output [S, D]; host sums the two
head-group partials per batch and adds bo.

Per-core device pipeline (all matmuls fp32r, 1 cycle/row):
  Phase A: from xT (host-pretransposed [D, S]) compute
           Q^T, K^T [1024, S] and V [S, 1024]; spill to DRAM scratch.
  Phase B: per head h: scores^T tiles [128 k, 512 q] = K_h Q_h^T,
           causal mask (additive, precomputed), exp (no max-subtract:
           scores are O(5), fp32 exp is safe), ctx^T accumulation
           C^T = V_h^T-blocks @ P^T, denominators via ones-matmul,
           normalize with reciprocal broadcast (PE outer product).
  Phase C: out_partial = C @ Wo_slice via C^T blocks as lhsT.
"""

import numpy as np

import concourse.bass as bass
import concourse.mybir as mybir
import concourse.tile as tile
from concourse import bacc
from concourse.bass_utils import run_bass_kernel_spmd

B = 4
S = 2048
D = 2048
H = 16
DH = 128
HPC = 8          # heads per core
DHG = HPC * DH   # 1024: head-group width per core
KT = D // 128    # 16 k-tiles over the model dim
ST = S // 128    # 16 s-tiles
QC = S // 512    # 4 q-chunks
SCALE = 1.0 / np.sqrt(DH)
NEG = -1.0e30

F32 = mybir.dt.float32
F32R = mybir.dt.float32r


def _build_nc():
    nc = bacc.Bacc(None, target_bir_lowering=False)

    xT = nc.declare_dram_parameter("xT", [D, S], F32, isOutput=False)
    # wq/wk host-pregathered to [HPC*128, KT*128]: row t*128+p, col n*128+m
    # = Wq[n*128+p, t*128+m] so each head-tile's weights DMA contiguously
    wq = nc.declare_dram_parameter("wq", [DHG, D], F32, isOutput=False)
    wk = nc.declare_dram_parameter("wk", [DHG, D], F32, isOutput=False)
    wv = nc.declare_dram_parameter("wv", [D, DHG], F32, isOutput=False)
    wo = nc.declare_dram_parameter("wo", [DHG, D], F32, isOutput=False)
    bqT = nc.declare_dram_parameter("bqT", [128, HPC], F32, isOutput=False)
    bkT = nc.declare_dram_parameter("bkT", [128, HPC], F32, isOutput=False)
    bvb = nc.declare_dram_parameter("bvb", [128, DHG], F32, isOutput=False)
    cmask = nc.declare_dram_parameter("cmask", [128, 896], F32, isOutput=False)
    out = nc.declare_dram_parameter("out", [S, D], F32, isOutput=True)

    with tile.TileContext(nc) as tc:
        _emit(nc, tc, xT, wq, wk, wv, wo, bqT, bkT, bvb, cmask, out)
    nc.compile()
    return nc


def _emit(nc, tc, xT, wq, wk, wv, wo, bqT, bkT, bvb, cmask, out):
    with (
        tc.tile_pool(name="const", bufs=1) as const,
        tc.tile_pool(name="dram", bufs=1, space="DRAM") as dram,
        tc.tile_pool(name="qkpre", bufs=2) as qkpre,
    ):
        qt_d = dram.tile([DHG, S], F32R)
        kt_d = dram.tile([DHG, S], F32R)
        v_d = dram.tile([S, DHG], F32R)
        ct_d = dram.tile([DHG, S], F32R)

        cm_sb = const.tile([128, 896], F32)
        nc.sync.dma_start(out=cm_sb, in_=cmask[:, :])
        bq_sb = const.tile([128, HPC], F32)
        nc.sync.dma_start(out=bq_sb, in_=bqT[:, :])
        bk_sb = const.tile([128, HPC], F32)
        nc.sync.dma_start(out=bk_sb, in_=bkT[:, :])
        bv_sb = const.tile([128, DHG], F32)
        nc.sync.dma_start(out=bv_sb, in_=bvb[:, :])
        ones_f32 = const.tile([128, 128], F32)
        nc.vector.memset(ones_f32, 1.0)
        ones128 = const.tile([128, 128], F32R)
        nc.vector.tensor_copy(out=ones128, in_=ones_f32)

        v_r = v_d[:, :].rearrange("(n p) d -> p n d", p=128)
        ct_r = ct_d[:, :].rearrange("(n p) m -> p n m", p=128)
        head0 = {}   # prefetched head-0 tiles, loaded during phase A
        ct_pre = {}  # prefetched phase-C ct tiles, loaded during phase B

        # ---------------- Phase A: projections, spilled to DRAM -------------
        wv_r = wv.bitcast(F32R).rearrange("(n p) m -> p n m", p=128)

        with (
            tc.tile_pool(name="xts", bufs=3) as xtp,
            tc.tile_pool(name="wqk", bufs=2) as wqk,
            tc.tile_pool(name="wvp", bufs=2) as wvp,
            tc.tile_pool(name="apsum", bufs=8, space="PSUM") as aps,
            tc.tile_pool(name="astage", bufs=4) as ast,
        ):
            for sh in range(2):
                s0 = sh * (S // 2)
                # weight tiles prefetched (depth 2) ahead of the bulk xT DMAs
                seq = [(w, b, dst, t)
                       for w, b, dst in ((wq, bq_sb, qt_d), (wk, bk_sb, kt_d))
                       for t in range(HPC)]
                w_tiles = {}

                def w_prefetch(i):
                    if i < len(seq):
                        w, _, _, t = seq[i]
                        w_sb = wqk.tile([128, KT, 128], F32R, tag="wqk",
                                        name=f"w_sb{i % 2}")
                        nc.sync.dma_start(
                            out=w_sb,
                            in_=w[t * 128 : (t + 1) * 128, :]
                            .rearrange("p (n m) -> p n m", m=128)
                            .bitcast(F32R),
                        )
                        w_tiles[i] = w_sb

                w_prefetch(0)

                # xT half as two sub-tiles of 8 k-tiles each (bufs=3 lets the
                # next half's first sub-tile prefetch during this half).
                xt_lo = xtp.tile([128, 8, S // 2], F32R, tag="xts")
                xt_hi = xtp.tile([128, 8, S // 2], F32R, tag="xts")

                def xt_blk(kd):
                    t = xt_lo if kd < 8 else xt_hi
                    return t[:, kd % 8, :]

                for kd in range(KT):
                    nc.sync.dma_start(
                        out=xt_blk(kd),
                        in_=xT[kd * 128 : (kd + 1) * 128, s0 : s0 + S // 2].bitcast(F32R),
                    )

                # Q^T and K^T: psum[dh 128, s 512] = sum_kd Wblk^T @ xTblk
                for i, (w, b_sb, dst, t) in enumerate(seq):
                    w_sb = w_tiles.pop(i)
                    w_prefetch(i + 1)
                    for sc in range(2):
                        psum = aps.tile([128, 512], F32, tag="apsum", name="qk_ps")
                        for kd in range(KT):
                            nc.tensor.matmul(
                                psum,
                                w_sb[:, kd, :],
                                xt_blk(kd)[:, sc * 512 : (sc + 1) * 512],
                                start=(kd == 0),
                                stop=(kd == KT - 1),
                            )
                        stg = ast.tile([128, 512], F32R, tag="astage")
                        nc.vector.tensor_scalar_add(
                            out=stg, in0=psum, scalar1=b_sb[:, t : t + 1]
                        )
                        nc.gpsimd.dma_start(
                            out=dst[
                                t * 128 : (t + 1) * 128,
                                s0 + sc * 512 : s0 + (sc + 1) * 512,
                            ],
                            in_=stg,
                        )

                if sh == 1:
                    qt0 = qkpre.tile([128, S], F32R, tag="qt", name="qt0")
                    nc.sync.dma_start(out=qt0, in_=qt_d[0:128, :])
                    kt0 = qkpre.tile([128, S], F32R, tag="kt", name="kt0")
                    nc.sync.dma_start(out=kt0, in_=kt_d[0:128, :])
                    head0["qt"] = qt0
                    head0["kt"] = kt0

                # V: psum[s 128, dh 512] = sum_kd xTblk^T @ Wvblk.
                # wv streamed two k-tiles per DMA; 4 s-tiles accumulate at once.
                for t2 in range(2):
                    # wv half-chunks (8 k-tiles each), double-buffered; all 8
                    # s-tiles of this half accumulate in one kd sweep
                    wv_lo = wvp.tile([128, 8, 512], F32R, tag="wvp", name="wv_lo")
                    nc.sync.dma_start(
                        out=wv_lo, in_=wv_r[:, 0:8, t2 * 512 : (t2 + 1) * 512]
                    )
                    wv_hi = wvp.tile([128, 8, 512], F32R, tag="wvp", name="wv_hi")
                    nc.sync.dma_start(
                        out=wv_hi, in_=wv_r[:, 8:16, t2 * 512 : (t2 + 1) * 512]
                    )
                    psums = [
                        aps.tile([128, 512], F32, tag="apsum", name=f"vps{si}")
                        for si in range(8)
                    ]
                    for kd in range(KT):
                        wv_blk = wv_lo if kd < 8 else wv_hi
                        for si in range(8):
                            nc.tensor.matmul(
                                psums[si],
                                xt_blk(kd)[:, si * 128 : (si + 1) * 128],
                                wv_blk[:, kd % 8, :],
                                start=(kd == 0),
                                stop=(kd == KT - 1),
                            )
                    for si in range(8):
                        stg = ast.tile([128, 512], F32R, tag="astage")
                        nc.vector.tensor_tensor(
                            out=stg,
                            in0=psums[si],
                            in1=bv_sb[:, t2 * 512 : (t2 + 1) * 512],
                            op=mybir.AluOpType.add,
                        )
                        nc.gpsimd.dma_start(
                            out=v_d[
                                s0 + si * 128 : s0 + (si + 1) * 128,
                                t2 * 512 : (t2 + 1) * 512,
                            ],
                            in_=stg,
                        )

        # ---------------- Phase B: per-head attention ------------------------
        wo_r = wo.bitcast(F32R).rearrange("(n p) m -> p n m", p=128)
        wop_cm = tc.tile_pool(name="wop", bufs=1)
        wop = wop_cm.__enter__()
        ctin_cm = tc.tile_pool(name="ctin", bufs=3)
        ctin = ctin_cm.__enter__()
        with (
            tc.tile_pool(name="vpool", bufs=2) as vpool,
            tc.tile_pool(name="ct", bufs=4) as ctpool,
            tc.tile_pool(name="ptile", bufs=6) as ppool,
            tc.tile_pool(name="msk", bufs=3) as mpool,
            tc.tile_pool(name="rcp", bufs=2) as rcpool,
            tc.tile_pool(name="pscore", bufs=3, space="PSUM") as pscore,
            tc.tile_pool(name="pctx", bufs=2, space="PSUM") as pctx,
            tc.tile_pool(name="psum2", bufs=2, space="PSUM") as psums,
        ):
            wo_sb = wop.tile([128, HPC, D], F32R)
            for h in range(HPC):
                if h == 0:
                    qt_sb = head0["qt"]
                    kt_sb = head0["kt"]
                else:
                    qt_sb = qkpre.tile([128, S], F32R, tag="qt", name="qt_sb")
                    nc.sync.dma_start(out=qt_sb, in_=qt_d[h * 128 : (h + 1) * 128, :])
                    kt_sb = qkpre.tile([128, S], F32R, tag="kt", name="kt_sb")
                    nc.sync.dma_start(out=kt_sb, in_=kt_d[h * 128 : (h + 1) * 128, :])
                v_sb = vpool.tile([128, ST, 128], F32R, tag="v", name="v_sb")
                nc.sync.dma_start(out=v_sb, in_=v_r[:, :, h * 128 : (h + 1) * 128])
                # spread the 8MB Wo load through phase B on the idle sync queue
                nc.sync.dma_start(out=wo_sb[:, h, :], in_=wo_r[:, h, :])

                for qc in range(QC):
                    nkt = 4 * qc + 4
                    # diagonal tiles first: their longer PE->DVE->ACT chains
                    # start early and overlap with the full tiles' stream
                    order = list(range(4 * qc, nkt)) + list(range(4 * qc))
                    psum_c = pctx.tile([128, 512], F32)
                    psum_s = psums.tile([128, 512], F32)

                    def scores(kt_i):
                        # diagonal tile j has valid columns only at qq >= 128j:
                        # compute just that [128, 512-128j] strip
                        j = kt_i - 4 * qc
                        off = 128 * j if j > 0 else 0
                        ps_t = pscore.tile([128, 512], F32, tag="ps_t")
                        nc.tensor.matmul(
                            ps_t[:, off:],
                            kt_sb[:, kt_i * 128 : (kt_i + 1) * 128],
                            qt_sb[:, qc * 512 + off : (qc + 1) * 512],
                            start=True,
                            stop=True,
                        )
                        p_t = ppool.tile([128, 512], F32R, tag="p_t")
                        if j >= 0:
                            msk = mpool.tile([128, 512], F32, tag="msk")
                            nc.vector.tensor_tensor(
                                out=msk[:, off:],
                                in0=ps_t[:, off:],
                                in1=cm_sb[:, 384 : 896 - off],
                                op=mybir.AluOpType.add,
                            )
                            src = msk
                        else:
                            src = ps_t
                        nc.scalar.activation(
                            out=p_t[:, off:],
                            in_=src[:, off:],
                            func=mybir.ActivationFunctionType.Exp,
                            scale=float(SCALE),
                        )
                        return p_t, off

                    def ctx(idx, kt_i, p_t, off):
                        nc.tensor.matmul(
                            psum_c[:, off:],
                            v_sb[:, kt_i, :],
                            p_t[:, off:],
                            start=(idx == 0),
                            stop=(idx == nkt - 1),
                        )
                        # every psum_s row accumulates the per-q denominator
                        nc.tensor.matmul(
                            psum_s[:, off:],
                            ones128,
                            p_t[:, off:],
                            start=(idx == 0),
                            stop=(idx == nkt - 1),
                        )

                    # software-pipeline scores/exp ahead of ctx by one tile
                    prev = None
                    for idx, kt_i in enumerate(order):
                        p_t, off = scores(kt_i)
                        if prev is not None:
                            ctx(idx - 1, prev[0], prev[1], prev[2])
                        prev = (kt_i, p_t, off)
                    ctx(nkt - 1, prev[0], prev[1], prev[2])

                    recip = rcpool.tile([128, 512], F32, tag="rcp")
                    nc.vector.reciprocal_approx_fast(out=recip, in_=psum_s)
                    ct = ctpool.tile([128, 512], F32R, tag="ct")
                    nc.vector.tensor_tensor(
                        out=ct,
                        in0=psum_c,
                        in1=recip,
                        op=mybir.AluOpType.mult,
                    )
                    nc.gpsimd.dma_start(
                        out=ct_d[h * 128 : (h + 1) * 128, qc * 512 : (qc + 1) * 512],
                        in_=ct,
                    )
                    if h == HPC - 1 and qc < 2:
                        pre = ctin.tile([128, HPC, 128], F32R, tag="ctin",
                                        name=f"ctpre{qc}")
                        nc.sync.dma_start(
                            out=pre,
                            in_=ct_r[:, :, qc * 4 * 128 : (qc * 4 + 1) * 128],
                        )
                        ct_pre[qc * 4] = pre

        # ---------------- Phase C: output projection -------------------------
        with (
            tc.tile_pool(name="opsum", bufs=4, space="PSUM") as ops,
            tc.tile_pool(name="ostage", bufs=4) as ost,
        ):
            for st in range(ST):
                if st in ct_pre:
                    ct_sb = ct_pre[st]
                else:
                    ct_sb = ctin.tile([128, HPC, 128], F32R, tag="ctin",
                                      name="ct_sb")
                    nc.sync.dma_start(
                        out=ct_sb, in_=ct_r[:, :, st * 128 : (st + 1) * 128]
                    )
                for ncol in range(4):
                    psum = ops.tile([128, 512], F32)
                    for hh in range(HPC):
                        nc.tensor.matmul(
                            psum,
                            ct_sb[:, hh, :],
                            wo_sb[:, hh, ncol * 512 : (ncol + 1) * 512],
                            start=(hh == 0),
                            stop=(hh == HPC - 1),
                        )
                    o_sb = ost.tile([128, 512], F32, tag="ostage")
                    nc.scalar.activation(
                        out=o_sb, in_=psum, func=mybir.ActivationFunctionType.Copy
                    )
                    nc.gpsimd.dma_start(
                        out=out[
                            st * 128 : (st + 1) * 128,
                            ncol * 512 : (ncol + 1) * 512,
                        ],
                        in_=o_sb,
                    )
        ctin_cm.__exit__(None, None, None)
        wop_cm.__exit__(None, None, None)


ctpool_tiles = {}

_NC = None


def _get_nc():
    global _NC
    if _NC is None:
        ctpool_tiles.clear()
        _NC = _build_nc()
    return _NC


def _host_prep(input_sequences, Wq, bq, Wk, bk, Wv, bv, Wo, bo):
    """Build per-core input maps."""
    x = np.asarray(input_sequences, dtype=np.float32)
    cm = np.full((128, 896), NEG, dtype=np.float32)
    kk = np.arange(128)[:, None]
    uu = np.arange(896)[None, :]
    cm[kk <= uu - 384] = 0.0

    in_maps = []
    for c in range(8):
        b, g = divmod(c, 2)
        sl = slice(g * DHG, (g + 1) * DHG)
        wq_c = np.ascontiguousarray(
            np.asarray(Wq[:, sl], dtype=np.float32)
            .reshape(KT, 128, HPC, 128).transpose(2, 1, 0, 3).reshape(DHG, D)
        )
        wk_c = np.ascontiguousarray(
            np.asarray(Wk[:, sl], dtype=np.float32)
            .reshape(KT, 128, HPC, 128).transpose(2, 1, 0, 3).reshape(DHG, D)
        )
        wv_c = np.ascontiguousarray(Wv[:, sl], dtype=np.float32)
        wo_c = np.ascontiguousarray(Wo[sl, :], dtype=np.float32)
        in_maps.append({
            "xT": np.ascontiguousarray(x[b].T),
            "wq": wq_c,
            "wk": wk_c,
            "wv": wv_c,
            "wo": wo_c,
            "bqT": np.ascontiguousarray(
                np.asarray(bq[sl], dtype=np.float32).reshape(HPC, 128).T
            ),
            "bkT": np.ascontiguousarray(
                np.asarray(bk[sl], dtype=np.float32).reshape(HPC, 128).T
            ),
            "bvb": np.ascontiguousarray(
                np.broadcast_to(np.asarray(bv[sl], dtype=np.float32), (128, DHG))
            ),
            "cmask": cm,
        })
    return in_maps


def kernel(input_sequences, Wq, bq, Wk, bk, Wv, bv, Wo, bo, _trace=False):
    nc = _get_nc()
    in_maps = _host_prep(input_sequences, Wq, bq, Wk, bk, Wv, bv, Wo, bo)
    res = run_bass_kernel_spmd(nc, in_maps, list(range(8)), trace=_trace)
    bo32 = np.asarray(bo, dtype=np.float32)
    out = np.empty((B, S, D), dtype=np.float32)
    for b in range(B):
        out[b] = res.results[2 * b]["out"] + res.results[2 * b + 1]["out"] + bo32
    if _trace:
        kernel.last_exec_time_ns = res.exec_time_ns
    return out



# revision 10
# speedup vs baseline: 1.1561x; 1.1561x over previous
"""Causal multi-head attention on 8 trn2 NeuronCores.

Problem: B=4, S=2048, D=2048, H=16 heads, head_dim=128, causal softmax,
torch-style Linear projections (W stored [in, out]).

Sharding: core c handles batch b = c//2 and head-group g = c%2
(8 heads = 1024 output columns of Wq/Wk/Wv, 1024 rows of Wo).
Each core produces a partial output [S, D]; host sums the two
head-group partials per batch and adds bo.

v2 design (all-SBUF-resident, bf16, interleaved):
  - xT resident in SBUF as bf16 [128, 16kd, 2048s] (64KB/partition).
  - V for all heads computed upfront, resident bf16 [128, 16st, 1024dh].
  - Q^T/K^T computed per-head just-in-time into rotating [128, 2048]
    bf16 tiles; head h+1's projection matmuls are interleaved into the
    PE queue during head h's attention so the PE never waits on ACT exp.
  - Scores^T tiles [128k, <=512q] -> exp (ACT, scale folded) -> binary
    causal mask-mult on the first 128 cols of diagonal strips (DVE) ->
    ctx^T and denominator accumulation on PE -> reciprocal+normalize
    (DVE) into resident ct bf16.
  - Output projection from resident ct with streamed Wo chunks.
  - No DRAM scratch at all; total DMA ~32MB/core.
"""

import numpy as np
import ml_dtypes

import concourse.bass as bass
import concourse.mybir as mybir
import concourse.tile as tile
from concourse import bacc
from concourse.bass_utils import run_bass_kernel_spmd

B = 4
S = 2048
D = 2048
H = 16
DH = 128
HPC = 8          # heads per core
DHG = HPC * DH   # 1024: head-group width per core
KT = D // 128    # 16 k-tiles over the model dim
ST = S // 128    # 16 s-tiles
QC = S // 512    # 4 q-chunks
SCALE = 1.0 / np.sqrt(DH)

F32 = mybir.dt.float32
BF16 = mybir.dt.bfloat16
BF16_NP = ml_dtypes.bfloat16


def _build_nc():
    nc = bacc.Bacc(None, target_bir_lowering=False)

    xT = nc.declare_dram_parameter("xT", [D, S], BF16, isOutput=False)
    # wq/wk host-pregathered to [HPC*128, KT*128]: row t*128+p, col n*128+m
    # = Wq[n*128+p, t*128+m] so each head-tile's weights DMA contiguously
    wq = nc.declare_dram_parameter("wq", [DHG, D], BF16, isOutput=False)
    wk = nc.declare_dram_parameter("wk", [DHG, D], BF16, isOutput=False)
    wv = nc.declare_dram_parameter("wv", [D, DHG], BF16, isOutput=False)
    # wo host-prepped to [128, HPC*D]: [p, h, m] = Wo[g*DHG + h*128 + p, m]
    wo = nc.declare_dram_parameter("wo", [128, HPC * D], BF16, isOutput=False)
    bqT = nc.declare_dram_parameter("bqT", [128, HPC], F32, isOutput=False)
    bkT = nc.declare_dram_parameter("bkT", [128, HPC], F32, isOutput=False)
    bvb = nc.declare_dram_parameter("bvb", [128, DHG], F32, isOutput=False)
    # binary causal mask for the first 128 cols of a diagonal strip:
    # mb[p, u] = 1.0 if p <= u else 0.0
    mb = nc.declare_dram_parameter("mb", [128, 128], BF16, isOutput=False)
    out = nc.declare_dram_parameter("out", [S, D], F32, isOutput=True)

    with tile.TileContext(nc) as tc:
        _emit(nc, tc, xT, wq, wk, wv, wo, bqT, bkT, bvb, mb, out)
    nc.compile()
    return nc


def _emit(nc, tc, xT, wq, wk, wv, wo, bqT, bkT, bvb, mb, out):
    fadd = mybir.AluOpType.add
    fmul = mybir.AluOpType.mult
    with (
        tc.tile_pool(name="const", bufs=1) as const,
        tc.tile_pool(name="resident", bufs=1) as res,
        tc.tile_pool(name="qk", bufs=2) as qkp,
        tc.tile_pool(name="wqk", bufs=4) as wqkp,
        tc.tile_pool(name="wbig", bufs=2) as wbig,
        tc.tile_pool(name="ptile", bufs=4) as ppool,
        tc.tile_pool(name="rcp", bufs=1) as rcpool,
        tc.tile_pool(name="ostage", bufs=2) as ost,
        tc.tile_pool(name="ps8", bufs=2, space="PSUM") as ps8,
    ):
        # ---------------- constants ----------------
        mb_sb = const.tile([128, 128], BF16)
        nc.sync.dma_start(out=mb_sb, in_=mb[:, :])
        bq_sb = const.tile([128, HPC], F32)
        nc.sync.dma_start(out=bq_sb, in_=bqT[:, :])
        bk_sb = const.tile([128, HPC], F32)
        nc.sync.dma_start(out=bk_sb, in_=bkT[:, :])
        bv_sb = const.tile([128, DHG], F32)
        nc.sync.dma_start(out=bv_sb, in_=bvb[:, :])
        ones_f32 = const.tile([128, 128], F32)
        nc.vector.memset(ones_f32, 1.0)
        ones_bf = const.tile([128, 128], BF16)
        nc.vector.tensor_copy(out=ones_bf, in_=ones_f32)

        # ---------------- residents ----------------
        xt = res.tile([128, KT, S], BF16)          # 64KB/part
        v_sb = res.tile([128, ST, DHG], BF16)      # 32KB/part
        ct = res.tile([128, HPC, S], BF16)         # 32KB/part

        xT_r = xT.rearrange("(n p) s -> p n s", p=128)
        wv_r = wv.rearrange("(n p) m -> p n m", p=128)
        wo_r = wo.rearrange("p (h m) -> p h m", m=D)

        # DMA order matters: stage V consumes xt quarter 0 + wv half 0
        # first, so land those before the bulk of xT
        nc.sync.dma_start(out=xt[:, :, 0:512], in_=xT_r[:, :, 0:512])
        wv_hs = []
        for t2 in range(2):
            wv_h = wbig.tile([128, KT, 512], BF16, tag="wbig", name=f"wv_h{t2}")
            nc.sync.dma_start(
                out=wv_h, in_=wv_r[:, :, t2 * 512 : (t2 + 1) * 512]
            )
            wv_hs.append(wv_h)
        for sq in range(1, 4):
            nc.sync.dma_start(
                out=xt[:, :, sq * 512 : (sq + 1) * 512],
                in_=xT_r[:, :, sq * 512 : (sq + 1) * 512],
            )

        # ---------------- stage V: all heads ----------------
        # psum[s 128, dh 512] = sum_kd xt_blk^T @ wv_blk, two dh halves
        for t2 in range(2):
            wv_h = wv_hs[t2]
            for st in range(ST):
                psum = ps8.tile([128, 512], F32, tag="c" if st % 2 == 0 else "s",
                                name="v_ps")
                for kd in range(KT):
                    nc.tensor.matmul(
                        psum,
                        xt[:, kd, st * 128 : (st + 1) * 128],
                        wv_h[:, kd, :],
                        start=(kd == 0),
                        stop=(kd == KT - 1),
                    )
                nc.vector.tensor_tensor(
                    out=v_sb[:, st, t2 * 512 : (t2 + 1) * 512],
                    in0=psum,
                    in1=bv_sb[:, t2 * 512 : (t2 + 1) * 512],
                    op=fadd,
                )

        # ---------------- per-head QK projection (emitted lazily) --------
        def prep_w(h):
            """Issue the Wq/Wk head-tile DMAs well ahead of use."""
            w_sbs = []
            for w in (wq, wk):
                w_sb = wqkp.tile([128, KT, 128], BF16, tag="wqk",
                                 name=f"w_sb{h % 2}")
                nc.sync.dma_start(
                    out=w_sb,
                    in_=w[h * 128 : (h + 1) * 128, :]
                    .rearrange("p (n m) -> p n m", m=128),
                )
                w_sbs.append(w_sb)
            return w_sbs

        def make_qk(h, w_sbs):
            """Generator: emits QK_h matmuls a few at a time; returns tiles
            immediately (they are filled as the generator is drained)."""
            qt_t = qkp.tile([128, S], BF16, tag="qt", name=f"qt{h % 2}")
            kt_t = qkp.tile([128, S], BF16, tag="kt", name=f"kt{h % 2}")

            def gen():
                for w_sb, b_sb, dst in (
                    (w_sbs[0], bq_sb, qt_t),
                    (w_sbs[1], bk_sb, kt_t),
                ):
                    for sq in range(4):
                        psum = ps8.tile([128, 512], F32, tag="qk", name="qk_ps")
                        for kd in range(KT):
                            nc.tensor.matmul(
                                psum,
                                w_sb[:, kd, :],
                                xt[:, kd, sq * 512 : (sq + 1) * 512],
                                start=(kd == 0),
                                stop=(kd == KT - 1),
                            )
                            yield
                        nc.vector.tensor_scalar_add(
                            out=dst[:, sq * 512 : (sq + 1) * 512],
                            in0=psum,
                            scalar1=b_sb[:, h : h + 1],
                        )
                while True:
                    yield

            return qt_t, kt_t, gen()

        # ------------- attention per head, QK_{h+1} interleaved ----------
        def wo_load(ncol):
            wo_c = wbig.tile([128, HPC, 512], BF16, tag="wbig",
                             name=f"wo_c{ncol}")
            nc.sync.dma_start(
                out=wo_c, in_=wo_r[:, :, ncol * 512 : (ncol + 1) * 512]
            )
            return wo_c

        wo_chunks = []
        w_pre = {0: prep_w(0), 1: prep_w(1)}
        qt_cur, kt_cur, g0 = make_qk(0, w_pre.pop(0))
        for _ in range(140):
            next(g0)

        for h in range(HPC):
            if h + 2 < HPC:
                w_pre[h + 2] = prep_w(h + 2)
            if h + 1 < HPC:
                qt_nxt, kt_nxt, gnxt = make_qk(h + 1, w_pre.pop(h + 1))
            else:
                qt_nxt = kt_nxt = gnxt = None

            def fill(n):
                if gnxt is not None:
                    for _ in range(n):
                        next(gnxt)

            if h == HPC - 1:
                # last head: prefetch first Wo chunks for phase C instead
                wo_chunks.append(wo_load(0))
                wo_chunks.append(wo_load(1))

            for qc in range(QC):
                nkt = 4 * qc + 4
                # diagonal tiles first: their longer PE->ACT->DVE chains
                # start early and overlap with the full tiles' stream
                order = list(range(4 * qc, nkt)) + list(range(4 * qc))
                psum_c = ps8.tile([128, 512], F32, tag="c", name="psum_c")
                psum_s = ps8.tile([128, 512], F32, tag="s", name="psum_s")

                def scores(kt_i):
                    # diagonal tile j has valid columns only at qq >= 128j:
                    # compute just that [128, 512-128j] strip
                    j = kt_i - 4 * qc
                    off = 128 * j if j > 0 else 0
                    ps_t = ps8.tile([128, 512], F32, tag="st", name="ps_t")
                    nc.tensor.matmul(
                        ps_t[:, off:],
                        kt_cur[:, kt_i * 128 : (kt_i + 1) * 128],
                        qt_cur[:, qc * 512 + off : (qc + 1) * 512],
                        start=True,
                        stop=True,
                    )
                    p_t = ppool.tile([128, 512], BF16, tag="p_t")
                    nc.scalar.activation(
                        out=p_t[:, off:],
                        in_=ps_t[:, off:],
                        func=mybir.ActivationFunctionType.Exp,
                        scale=float(SCALE),
                    )
                    if j >= 0:
                        # zero p where k > q: only possible in the first
                        # 128 columns of the strip
                        nc.vector.tensor_tensor(
                            out=p_t[:, off : off + 128],
                            in0=p_t[:, off : off + 128],
                            in1=mb_sb,
                            op=fmul,
                        )
                    return p_t, off

                def ctx(idx, p_t, off, kt_i):
                    nc.tensor.matmul(
                        psum_c[:, off:],
                        v_sb[:, kt_i, h * 128 : (h + 1) * 128],
                        p_t[:, off:],
                        start=(idx == 0),
                        stop=(idx == nkt - 1),
                    )
                    nc.tensor.matmul(
                        psum_s[:, off:],
                        ones_bf,
                        p_t[:, off:],
                        start=(idx == 0),
                        stop=(idx == nkt - 1),
                    )

                # software-pipeline scores/exp ahead of ctx by one tile;
                # pad the PE queue with next head's projection matmuls
                prev = None
                for idx, kt_i in enumerate(order):
                    p_t, off = scores(kt_i)
                    fill(3)
                    if prev is not None:
                        ctx(idx - 1, prev[0], prev[1], prev[2])
                    prev = (p_t, off, kt_i)
                ctx(nkt - 1, prev[0], prev[1], prev[2])
                fill(2)

                recip = rcpool.tile([128, 512], F32, tag="rcp")
                nc.vector.reciprocal_approx_fast(out=recip, in_=psum_s)
                nc.vector.tensor_tensor(
                    out=ct[:, h, qc * 512 : (qc + 1) * 512],
                    in0=psum_c,
                    in1=recip,
                    op=fmul,
                )
            fill(200)  # drain any remainder of QK_{h+1}
            qt_cur, kt_cur = qt_nxt, kt_nxt

        # ---------------- output projection -------------------------
        for ncol in range(4):
            if ncol + 2 < 4:
                wo_chunks.append(wo_load(ncol + 2))
            wo_c = wo_chunks[ncol]
            for st in range(ST):
                psum = ps8.tile([128, 512], F32,
                                tag=("qk", "st", "c", "s")[st % 4], name="o_ps")
                for hh in range(HPC):
                    nc.tensor.matmul(
                        psum,
                        ct[:, hh, st * 128 : (st + 1) * 128],
                        wo_c[:, hh, :],
                        start=(hh == 0),
                        stop=(hh == HPC - 1),
                    )
                o_sb = ost.tile([128, 512], F32, tag="ostage")
                nc.scalar.activation(
                    out=o_sb, in_=psum, func=mybir.ActivationFunctionType.Copy
                )
                nc.gpsimd.dma_start(
                    out=out[
                        st * 128 : (st + 1) * 128,
                        ncol * 512 : (ncol + 1) * 512,
                    ],
                    in_=o_sb,
                )


_NC = None


def _get_nc():
    global _NC
    if _NC is None:
        _NC = _build_nc()
    return _NC


def _host_prep(input_sequences, Wq, bq, Wk, bk, Wv, bv, Wo, bo):
    """Build per-core input maps."""
    x = np.asarray(input_sequences, dtype=np.float32)
    mbm = (np.arange(128)[:, None] <= np.arange(128)[None, :]).astype(BF16_NP)

    in_maps = []
    for c in range(8):
        b, g = divmod(c, 2)
        sl = slice(g * DHG, (g + 1) * DHG)
        wq_c = np.ascontiguousarray(
            np.asarray(Wq[:, sl], dtype=np.float32)
            .reshape(KT, 128, HPC, 128).transpose(2, 1, 0, 3).reshape(DHG, D)
        ).astype(BF16_NP)
        wk_c = np.ascontiguousarray(
            np.asarray(Wk[:, sl], dtype=np.float32)
            .reshape(KT, 128, HPC, 128).transpose(2, 1, 0, 3).reshape(DHG, D)
        ).astype(BF16_NP)
        wv_c = np.ascontiguousarray(np.asarray(Wv[:, sl], np.float32)).astype(BF16_NP)
        wo_c = np.ascontiguousarray(
            np.asarray(Wo[sl, :], dtype=np.float32)
            .reshape(HPC, 128, D).transpose(1, 0, 2).reshape(128, HPC * D)
        ).astype(BF16_NP)
        in_maps.append({
            "xT": np.ascontiguousarray(x[b].T).astype(BF16_NP),
            "wq": wq_c,
            "wk": wk_c,
            "wv": wv_c,
            "wo": wo_c,
            "bqT": np.ascontiguousarray(
                np.asarray(bq[sl], dtype=np.float32).reshape(HPC, 128).T
            ),
            "bkT": np.ascontiguousarray(
                np.asarray(bk[sl], dtype=np.float32).reshape(HPC, 128).T
            ),
            "bvb": np.ascontiguousarray(
                np.broadcast_to(np.asarray(bv[sl], dtype=np.float32), (128, DHG))
            ),
            "mb": mbm,
        })
    return in_maps


def kernel(input_sequences, Wq, bq, Wk, bk, Wv, bv, Wo, bo, _trace=False):
    nc = _get_nc()
    in_maps = _host_prep(input_sequences, Wq, bq, Wk, bk, Wv, bv, Wo, bo)
    res = run_bass_kernel_spmd(nc, in_maps, list(range(8)), trace=_trace)
    bo32 = np.asarray(bo, dtype=np.float32)
    out = np.empty((B, S, D), dtype=np.float32)
    for b in range(B):
        out[b] = res.results[2 * b]["out"] + res.results[2 * b + 1]["out"] + bo32
    if _trace:
        kernel.last_exec_time_ns = res.exec_time_ns
    return out


# revision 21
# speedup vs baseline: 1.1873x; 1.0270x over previous
"""Causal multi-head attention on 8 trn2 NeuronCores.

Problem: B=4, S=2048, D=2048, H=16 heads, head_dim=128, causal softmax,
torch-style Linear projections (W stored [in, out]).

Sharding: core c handles batch b = c//2 and head-group g = c%2
(8 heads = 1024 output columns of Wq/Wk/Wv, 1024 rows of Wo).
Each core produces a partial output [S, D]; host sums the two
head-group partials per batch and adds bo.

v2 design (all-SBUF-resident, bf16, interleaved):
  - xT resident in SBUF as bf16 [128, 16kd, 2048s] (64KB/partition).
  - V for all heads computed upfront, resident bf16 [128, 16st, 1024dh].
  - Q^T/K^T computed per-head just-in-time into rotating [128, 2048]
    bf16 tiles; head h+1's projection matmuls are interleaved into the
    PE queue during head h's attention so the PE never waits on ACT exp.
  - Scores^T tiles [128k, <=512q] -> exp (ACT, scale folded) -> binary
    causal mask-mult on the first 128 cols of diagonal strips (DVE) ->
    ctx^T and denominator accumulation on PE -> reciprocal+normalize
    (DVE) into resident ct bf16.
  - Output projection from resident ct with streamed Wo chunks.
  - No DRAM scratch at all; total DMA ~32MB/core.
"""

import numpy as np
import ml_dtypes

import concourse.bass as bass
import concourse.mybir as mybir
import concourse.tile as tile
from concourse import bacc
from concourse.bass_utils import run_bass_kernel_spmd

B = 4
S = 2048
D = 2048
H = 16
DH = 128
HPC = 8          # heads per core
DHG = HPC * DH   # 1024: head-group width per core
KT = D // 128    # 16 k-tiles over the model dim
ST = S // 128    # 16 s-tiles
QC = S // 512    # 4 q-chunks
SCALE = 1.0 / np.sqrt(DH)

F32 = mybir.dt.float32
BF16 = mybir.dt.bfloat16
BF16_NP = ml_dtypes.bfloat16


def _build_nc():
    nc = bacc.Bacc(None, target_bir_lowering=False)

    # all weight/activation params host-packed so each DMA reads fully
    # contiguous per-partition rows (large descriptors)
    # xT: row sq*128+p, col kd*512+u = x[sq*512+u, kd*128+p]
    xT = nc.declare_dram_parameter("xT", [4 * 128, KT * 512], BF16,
                                   isOutput=False)
    # wq/wk: row t*128+p, col n*128+m = Wq[n*128+p, t*128+m]
    wq = nc.declare_dram_parameter("wq", [DHG, D], BF16, isOutput=False)
    wk = nc.declare_dram_parameter("wk", [DHG, D], BF16, isOutput=False)
    # wv: row t2*128+p, col kd*512+m = Wv[kd*128+p, t2*512+m]
    wv = nc.declare_dram_parameter("wv", [2 * 128, KT * 512], BF16,
                                   isOutput=False)
    # wo: [p, ncol*4096 + h*512 + m] = Wo[g*DHG + h*128 + p, ncol*512 + m]
    wo = nc.declare_dram_parameter("wo", [128, HPC * D], BF16, isOutput=False)
    bqT = nc.declare_dram_parameter("bqT", [128, HPC], F32, isOutput=False)
    bkT = nc.declare_dram_parameter("bkT", [128, HPC], F32, isOutput=False)
    bvb = nc.declare_dram_parameter("bvb", [128, DHG], F32, isOutput=False)
    # binary causal mask for the first 128 cols of a diagonal strip:
    # mb[p, u] = 1.0 if p <= u else 0.0
    mb = nc.declare_dram_parameter("mb", [128, 128], BF16, isOutput=False)
    out = nc.declare_dram_parameter("out", [S, D], BF16, isOutput=True)

    with tile.TileContext(nc) as tc:
        _emit(nc, tc, xT, wq, wk, wv, wo, bqT, bkT, bvb, mb, out)
    nc.compile()
    return nc


def _emit(nc, tc, xT, wq, wk, wv, wo, bqT, bkT, bvb, mb, out):
    fadd = mybir.AluOpType.add
    fmul = mybir.AluOpType.mult
    with (
        tc.tile_pool(name="const", bufs=1) as const,
        tc.tile_pool(name="resident", bufs=1) as res,
        tc.tile_pool(name="qk", bufs=2) as qkp,
        tc.tile_pool(name="wqk", bufs=3) as wqkp,
        tc.tile_pool(name="wbig", bufs=4) as wbig,
        tc.tile_pool(name="ptile", bufs=4) as ppool,
        tc.tile_pool(name="rcp", bufs=1) as rcpool,
        tc.tile_pool(name="ostage", bufs=2) as ost,
        tc.tile_pool(name="ps8", bufs=2, space="PSUM") as ps8,
    ):
        # ---------------- constants ----------------
        mb_sb = const.tile([128, 128], BF16)
        nc.sync.dma_start(out=mb_sb, in_=mb[:, :])
        bq_sb = const.tile([128, HPC], F32)
        nc.sync.dma_start(out=bq_sb, in_=bqT[:, :])
        bk_sb = const.tile([128, HPC], F32)
        nc.sync.dma_start(out=bk_sb, in_=bkT[:, :])
        bv_sb = const.tile([128, DHG], F32)
        nc.sync.dma_start(out=bv_sb, in_=bvb[:, :])
        ones_f32 = const.tile([128, 128], F32)
        nc.vector.memset(ones_f32, 1.0)
        ones_bf = const.tile([128, 128], BF16)
        nc.vector.tensor_copy(out=ones_bf, in_=ones_f32)

        # ---------------- residents ----------------
        xt = res.tile([128, 4, KT, 512], BF16)     # 64KB/part, quarter-major
        v_sb = res.tile([128, ST, DHG], BF16)      # 32KB/part
        ct = res.tile([128, HPC, S], BF16)         # 32KB/part

        wo_r = wo.rearrange("p (c h m) -> p c h m", h=HPC, m=512)

        # stage V consumes wv(t2=0, kd 0..7) + xt quarter 0 first; xt goes
        # on the (otherwise idle) gpsimd DMA queue so both load in parallel
        wv_qs = {}
        for t2 in range(2):
            for kh in range(2):
                wv_q = wbig.tile([128, 8, 512], BF16, tag="wbig",
                                 name=f"wv_q{t2}{kh}")
                nc.sync.dma_start(
                    out=wv_q,
                    in_=wv[t2 * 128 : (t2 + 1) * 128,
                           kh * 4096 : (kh + 1) * 4096]
                    .rearrange("p (n m) -> p n m", m=512),
                )
                wv_qs[t2, kh] = wv_q
        for sq in range(4):
            nc.gpsimd.dma_start(
                out=xt[:, sq],
                in_=xT[sq * 128 : (sq + 1) * 128, :]
                .rearrange("p (n u) -> p n u", u=512),
            )

        # ---------------- stage V: all heads ----------------
        # psum[s 128, dh 512] = sum_kd xt_blk^T @ wv_blk, two dh halves
        for t2 in range(2):
            for st in range(ST):
                psum = ps8.tile([128, 512], F32, tag="c" if st % 2 == 0 else "s",
                                name="v_ps")
                for kd in range(KT):
                    nc.tensor.matmul(
                        psum,
                        xt[:, st // 4, kd, (st % 4) * 128 : (st % 4 + 1) * 128],
                        wv_qs[t2, kd // 8][:, kd % 8, :],
                        start=(kd == 0),
                        stop=(kd == KT - 1),
                    )
                nc.vector.tensor_tensor(
                    out=v_sb[:, st, t2 * 512 : (t2 + 1) * 512],
                    in0=psum,
                    in1=bv_sb[:, t2 * 512 : (t2 + 1) * 512],
                    op=fadd,
                )

        # ---------------- per-head QK projection (emitted lazily) --------
        def prep_w(h):
            """Issue the Wq/Wk head-tile DMAs well ahead of use."""
            w_sbs = []
            for w in (wq, wk):
                w_sb = wqkp.tile([128, KT, 128], BF16, tag="wqk",
                                 name=f"w_sb{h % 2}")
                nc.sync.dma_start(
                    out=w_sb,
                    in_=w[h * 128 : (h + 1) * 128, :]
                    .rearrange("p (n m) -> p n m", m=128),
                )
                w_sbs.append(w_sb)
            return w_sbs

        def make_qk(h, w_sbs):
            """Generator: emits QK_h matmuls a few at a time; returns tiles
            immediately (they are filled as the generator is drained)."""
            qt_t = qkp.tile([128, S], BF16, tag="qt", name=f"qt{h % 2}")
            kt_t = qkp.tile([128, S], BF16, tag="kt", name=f"kt{h % 2}")

            def gen():
                for w_sb, b_sb, dst in (
                    (w_sbs[0], bq_sb, qt_t),
                    (w_sbs[1], bk_sb, kt_t),
                ):
                    for sq in range(4):
                        psum = ps8.tile([128, 512], F32, tag="qk", name="qk_ps")
                        for kd in range(KT):
                            nc.tensor.matmul(
                                psum,
                                w_sb[:, kd, :],
                                xt[:, sq, kd, :],
                                start=(kd == 0),
                                stop=(kd == KT - 1),
                            )
                            yield
                        nc.vector.tensor_scalar_add(
                            out=dst[:, sq * 512 : (sq + 1) * 512],
                            in0=psum,
                            scalar1=b_sb[:, h : h + 1],
                        )
                while True:
                    yield

            return qt_t, kt_t, gen()

        # ------------- attention per head, QK_{h+1} interleaved ----------
        def wo_load(ncol):
            wo_c = wbig.tile([128, HPC, 512], BF16, tag="wbig",
                             name=f"wo_c{ncol}")
            nc.sync.dma_start(out=wo_c, in_=wo_r[:, ncol])
            return wo_c

        wo_chunks = []
        w_pre = {0: prep_w(0), 1: prep_w(1)}
        qt_cur, kt_cur, g0 = make_qk(0, w_pre.pop(0))
        for _ in range(140):
            next(g0)

        for h in range(HPC):
            if h + 2 < HPC:
                w_pre[h + 2] = prep_w(h + 2)
            if h + 1 < HPC:
                qt_nxt, kt_nxt, gnxt = make_qk(h + 1, w_pre.pop(h + 1))
            else:
                qt_nxt = kt_nxt = gnxt = None

            def fill(n):
                if gnxt is not None:
                    for _ in range(n):
                        next(gnxt)

            if h == HPC - 1:
                # last head: prefetch all Wo chunks for phase C instead
                for ncol in range(4):
                    wo_chunks.append(wo_load(ncol))

            for qc in range(QC):
                nkt = 4 * qc + 4
                # diagonal tiles first: their longer PE->ACT->DVE chains
                # start early and overlap with the full tiles' stream
                order = list(range(4 * qc, nkt)) + list(range(4 * qc))
                psum_c = ps8.tile([128, 512], F32, tag="c", name="psum_c")
                psum_s = ps8.tile([128, 512], F32, tag="s", name="psum_s")

                def scores(kt_i):
                    # diagonal tile j has valid columns only at qq >= 128j:
                    # compute just that [128, 512-128j] strip
                    j = kt_i - 4 * qc
                    off = 128 * j if j > 0 else 0
                    ps_t = ps8.tile([128, 512], F32, tag="st", name="ps_t")
                    nc.tensor.matmul(
                        ps_t[:, off:],
                        kt_cur[:, kt_i * 128 : (kt_i + 1) * 128],
                        qt_cur[:, qc * 512 + off : (qc + 1) * 512],
                        start=True,
                        stop=True,
                    )
                    p_t = ppool.tile([128, 512], BF16, tag="p_t")
                    nc.scalar.activation(
                        out=p_t[:, off:],
                        in_=ps_t[:, off:],
                        func=mybir.ActivationFunctionType.Exp,
                        scale=float(SCALE),
                    )
                    if j >= 0:
                        # zero p where k > q: only possible in the first
                        # 128 columns of the strip
                        nc.vector.tensor_tensor(
                            out=p_t[:, off : off + 128],
                            in0=p_t[:, off : off + 128],
                            in1=mb_sb,
                            op=fmul,
                        )
                    return p_t, off

                def ctx(idx, p_t, off, kt_i):
                    nc.tensor.matmul(
                        psum_c[:, off:],
                        v_sb[:, kt_i, h * 128 : (h + 1) * 128],
                        p_t[:, off:],
                        start=(idx == 0),
                        stop=(idx == nkt - 1),
                    )
                    nc.tensor.matmul(
                        psum_s[:, off:],
                        ones_bf,
                        p_t[:, off:],
                        start=(idx == 0),
                        stop=(idx == nkt - 1),
                    )

                # software-pipeline scores/exp ahead of ctx by one tile;
                # pad the PE queue with next head's projection matmuls
                prev = None
                for idx, kt_i in enumerate(order):
                    p_t, off = scores(kt_i)
                    fill(3)
                    if prev is not None:
                        ctx(idx - 1, prev[0], prev[1], prev[2])
                    prev = (p_t, off, kt_i)
                ctx(nkt - 1, prev[0], prev[1], prev[2])
                fill(2)

                recip = rcpool.tile([128, 512], F32, tag="rcp")
                nc.vector.reciprocal_approx_fast(out=recip, in_=psum_s)
                nc.vector.tensor_tensor(
                    out=ct[:, h, qc * 512 : (qc + 1) * 512],
                    in0=psum_c,
                    in1=recip,
                    op=fmul,
                )
            fill(200)  # drain any remainder of QK_{h+1}
            qt_cur, kt_cur = qt_nxt, kt_nxt

        # ---------------- output projection -------------------------
        # st-outer so each s-row-block [128, 2048] stores as one DMA of
        # fully contiguous 4KB rows
        out_r = out.rearrange("s (c m) -> s c m", m=512)
        for st in range(ST):
            o_sb = ost.tile([128, 4, 512], BF16, tag="ostage")
            for ncol in range(4):
                psum = ps8.tile([128, 512], F32,
                                tag=("qk", "st", "c", "s")[ncol], name="o_ps")
                for hh in range(HPC):
                    nc.tensor.matmul(
                        psum,
                        ct[:, hh, st * 128 : (st + 1) * 128],
                        wo_chunks[ncol][:, hh, :],
                        start=(hh == 0),
                        stop=(hh == HPC - 1),
                    )
                nc.scalar.activation(
                    out=o_sb[:, ncol, :],
                    in_=psum,
                    func=mybir.ActivationFunctionType.Copy,
                )
            nc.gpsimd.dma_start(
                out=out_r[st * 128 : (st + 1) * 128], in_=o_sb
            )


_NC = None


def _get_nc():
    global _NC
    if _NC is None:
        _NC = _build_nc()
    return _NC


def _host_prep(input_sequences, Wq, bq, Wk, bk, Wv, bv, Wo, bo):
    """Build per-core input maps."""
    x = np.asarray(input_sequences, dtype=np.float32)
    mbm = (np.arange(128)[:, None] <= np.arange(128)[None, :]).astype(BF16_NP)

    in_maps = []
    for c in range(8):
        b, g = divmod(c, 2)
        sl = slice(g * DHG, (g + 1) * DHG)
        wq_c = np.ascontiguousarray(
            np.asarray(Wq[:, sl], dtype=np.float32)
            .reshape(KT, 128, HPC, 128).transpose(2, 1, 0, 3).reshape(DHG, D)
        ).astype(BF16_NP)
        wk_c = np.ascontiguousarray(
            np.asarray(Wk[:, sl], dtype=np.float32)
            .reshape(KT, 128, HPC, 128).transpose(2, 1, 0, 3).reshape(DHG, D)
        ).astype(BF16_NP)
        # wv packed [t2*128+p, kd*512+m] = Wv[kd*128+p, t2*512+m]
        wv_c = np.ascontiguousarray(
            np.asarray(Wv[:, sl], dtype=np.float32)
            .reshape(KT, 128, 2, 512).transpose(2, 1, 0, 3).reshape(256, KT * 512)
        ).astype(BF16_NP)
        # wo packed [p, ncol*4096 + h*512 + m] = Wo[sl][h*128+p, ncol*512+m]
        wo_c = np.ascontiguousarray(
            np.asarray(Wo[sl, :], dtype=np.float32)
            .reshape(HPC, 128, 4, 512).transpose(1, 2, 0, 3).reshape(128, HPC * D)
        ).astype(BF16_NP)
        # xT packed [sq*128+p, kd*512+u] = x[sq*512+u, kd*128+p]
        xt_c = np.ascontiguousarray(
            x[b].reshape(4, 512, KT, 128).transpose(0, 3, 2, 1)
            .reshape(512, KT * 512)
        ).astype(BF16_NP)
        in_maps.append({
            "xT": xt_c,
            "wq": wq_c,
            "wk": wk_c,
            "wv": wv_c,
            "wo": wo_c,
            "bqT": np.ascontiguousarray(
                np.asarray(bq[sl], dtype=np.float32).reshape(HPC, 128).T
            ),
            "bkT": np.ascontiguousarray(
                np.asarray(bk[sl], dtype=np.float32).reshape(HPC, 128).T
            ),
            "bvb": np.ascontiguousarray(
                np.broadcast_to(np.asarray(bv[sl], dtype=np.float32), (128, DHG))
            ),
            "mb": mbm,
        })
    return in_maps


def kernel(input_sequences, Wq, bq, Wk, bk, Wv, bv, Wo, bo, _trace=False):
    nc = _get_nc()
    in_maps = _host_prep(input_sequences, Wq, bq, Wk, bk, Wv, bv, Wo, bo)
    res = run_bass_kernel_spmd(nc, in_maps, list(range(8)), trace=_trace)
    bo32 = np.asarray(bo, dtype=np.float32)
    out = np.empty((B, S, D), dtype=np.float32)
    for b in range(B):
        out[b] = (
            res.results[2 * b]["out"].astype(np.float32)
            + res.results[2 * b + 1]["out"].astype(np.float32)
            + bo32
        )
    if _trace:
        kernel.last_exec_time_ns = res.exec_time_ns
    return out


# revision 25
# speedup vs baseline: 1.2335x; 1.0389x over previous
"""Causal multi-head attention on 8 trn2 NeuronCores.

Problem: B=4, S=2048, D=2048, H=16 heads, head_dim=128, causal softmax,
torch-style Linear projections (W stored [in, out]).

Sharding: core c handles batch b = c//2 and head-group g = c%2
(8 heads = 1024 output columns of Wq/Wk/Wv, 1024 rows of Wo).
Each core produces a partial output [S, D]; host sums the two
head-group partials per batch and adds bo.

v2 design (all-SBUF-resident, bf16, interleaved):
  - xT resident in SBUF as bf16 [128, 16kd, 2048s] (64KB/partition).
  - V for all heads computed upfront, resident bf16 [128, 16st, 1024dh].
  - Q^T/K^T computed per-head just-in-time into rotating [128, 2048]
    bf16 tiles; head h+1's projection matmuls are interleaved into the
    PE queue during head h's attention so the PE never waits on ACT exp.
  - Scores^T tiles [128k, <=512q] -> exp (ACT, scale folded) -> binary
    causal mask-mult on the first 128 cols of diagonal strips (DVE) ->
    ctx^T and denominator accumulation on PE -> reciprocal+normalize
    (DVE) into resident ct bf16.
  - Output projection from resident ct with streamed Wo chunks.
  - No DRAM scratch at all; total DMA ~32MB/core.
"""

import numpy as np
import ml_dtypes

import concourse.bass as bass
import concourse.mybir as mybir
import concourse.tile as tile
from concourse import bacc
from concourse.bass_utils import run_bass_kernel_spmd

B = 4
S = 2048
D = 2048
H = 16
DH = 128
HPC = 8          # heads per core
DHG = HPC * DH   # 1024: head-group width per core
KT = D // 128    # 16 k-tiles over the model dim
ST = S // 128    # 16 s-tiles
QC = S // 512    # 4 q-chunks
SCALE = 1.0 / np.sqrt(DH)

F32 = mybir.dt.float32
BF16 = mybir.dt.bfloat16
BF16_NP = ml_dtypes.bfloat16


def _build_nc():
    nc = bacc.Bacc(None, target_bir_lowering=False)

    # all weight/activation params host-packed so each DMA reads fully
    # contiguous per-partition rows (large descriptors)
    # xT: row sq*128+p, col blk*2048 + kd*128 + u = x[sq*512+blk*128+u, kd*128+p]
    xT = nc.declare_dram_parameter("xT", [4 * 128, KT * 512], BF16,
                                   isOutput=False)
    # wq/wk: row t*128+p, col n*128+m = Wq[n*128+p, t*128+m]
    wq = nc.declare_dram_parameter("wq", [DHG, D], BF16, isOutput=False)
    wk = nc.declare_dram_parameter("wk", [DHG, D], BF16, isOutput=False)
    # wv: row t2*128+p, col kd*512+m = Wv[kd*128+p, t2*512+m]
    wv = nc.declare_dram_parameter("wv", [2 * 128, KT * 512], BF16,
                                   isOutput=False)
    # wo: [p, ncol*4096 + h*512 + m] = Wo[g*DHG + h*128 + p, ncol*512 + m]
    wo = nc.declare_dram_parameter("wo", [128, HPC * D], BF16, isOutput=False)
    bqT = nc.declare_dram_parameter("bqT", [128, HPC], F32, isOutput=False)
    bkT = nc.declare_dram_parameter("bkT", [128, HPC], F32, isOutput=False)
    bvb = nc.declare_dram_parameter("bvb", [128, DHG], F32, isOutput=False)
    # binary causal mask for the first 128 cols of a diagonal strip:
    # mb[p, u] = 1.0 if p <= u else 0.0
    mb = nc.declare_dram_parameter("mb", [128, 128], BF16, isOutput=False)
    out = nc.declare_dram_parameter("out", [S, D], BF16, isOutput=True)

    with tile.TileContext(nc) as tc:
        _emit(nc, tc, xT, wq, wk, wv, wo, bqT, bkT, bvb, mb, out)
    nc.compile()
    return nc


def _emit(nc, tc, xT, wq, wk, wv, wo, bqT, bkT, bvb, mb, out):
    fadd = mybir.AluOpType.add
    fmul = mybir.AluOpType.mult
    with (
        tc.tile_pool(name="const", bufs=1) as const,
        tc.tile_pool(name="resident", bufs=1) as res,
        tc.tile_pool(name="qk", bufs=2) as qkp,
        tc.tile_pool(name="wqk", bufs=3) as wqkp,
        tc.tile_pool(name="wbig", bufs=4) as wbig,
        tc.tile_pool(name="ptile", bufs=4) as ppool,
        tc.tile_pool(name="rcp", bufs=1) as rcpool,
        tc.tile_pool(name="ostage", bufs=2) as ost,
        tc.tile_pool(name="ps8", bufs=2, space="PSUM") as ps8,
    ):
        # ---------------- constants ----------------
        mb_sb = const.tile([128, 128], BF16)
        nc.sync.dma_start(out=mb_sb, in_=mb[:, :])
        bq_sb = const.tile([128, HPC], F32)
        nc.sync.dma_start(out=bq_sb, in_=bqT[:, :])
        bk_sb = const.tile([128, HPC], F32)
        nc.sync.dma_start(out=bk_sb, in_=bkT[:, :])
        bv_sb = const.tile([128, DHG], F32)
        nc.sync.dma_start(out=bv_sb, in_=bvb[:, :])
        ones_f32 = const.tile([128, 128], F32)
        nc.vector.memset(ones_f32, 1.0)
        ones_bf = const.tile([128, 128], BF16)
        nc.vector.tensor_copy(out=ones_bf, in_=ones_f32)

        # ---------------- residents ----------------
        # xt block-major: [p, sq, blk, kd, u]; an s-tile st=(sq*4+blk) is a
        # contiguous 4KB run per partition
        xt = res.tile([128, 4, 4, KT, 128], BF16)  # 64KB/part
        v_sb = res.tile([128, ST, DHG], BF16)      # 32KB/part
        ct = res.tile([128, HPC, S], BF16)         # 32KB/part

        wo_r = wo.rearrange("p (c h m) -> p c h m", h=HPC, m=512)
        xT_r4 = [
            xT[sq * 128 : (sq + 1) * 128, :]
            .rearrange("p (b n u) -> p b n u", b=4, u=128)
            for sq in range(4)
        ]

        # priority DMA order: the first V s-tile needs wv(t2=0) + xt block
        # (0,0) only, so land those first, then stream the rest
        wv_qs = {}

        def wv_load(t2, kh):
            wv_q = wbig.tile([128, 8, 512], BF16, tag="wbig",
                             name=f"wv_q{t2}{kh}")
            nc.sync.dma_start(
                out=wv_q,
                in_=wv[t2 * 128 : (t2 + 1) * 128,
                       kh * 4096 : (kh + 1) * 4096]
                .rearrange("p (n m) -> p n m", m=512),
            )
            wv_qs[t2, kh] = wv_q

        wv_load(0, 0)
        nc.sync.dma_start(out=xt[:, 0, 0], in_=xT_r4[0][:, 0])
        wv_load(0, 1)
        for blk in range(1, 4):
            nc.sync.dma_start(out=xt[:, 0, blk], in_=xT_r4[0][:, blk])
        wv_load(1, 0)
        wv_load(1, 1)
        for sq in range(1, 4):
            nc.sync.dma_start(out=xt[:, sq], in_=xT_r4[sq])

        # ---------------- stage V: all heads ----------------
        # psum[s 128, dh 512] = sum_kd xt_blk^T @ wv_blk, two dh halves
        for t2 in range(2):
            for st in range(ST):
                psum = ps8.tile([128, 512], F32, tag="c" if st % 2 == 0 else "s",
                                name="v_ps")
                for kd in range(KT):
                    nc.tensor.matmul(
                        psum,
                        xt[:, st // 4, st % 4, kd, :],
                        wv_qs[t2, kd // 8][:, kd % 8, :],
                        start=(kd == 0),
                        stop=(kd == KT - 1),
                    )
                nc.vector.tensor_tensor(
                    out=v_sb[:, st, t2 * 512 : (t2 + 1) * 512],
                    in0=psum,
                    in1=bv_sb[:, t2 * 512 : (t2 + 1) * 512],
                    op=fadd,
                )

        # ---------------- per-head QK projection (emitted lazily) --------
        def prep_w(h):
            """Issue the Wq/Wk head-tile DMAs well ahead of use."""
            w_sbs = []
            for w in (wq, wk):
                w_sb = wqkp.tile([128, KT, 128], BF16, tag="wqk",
                                 name=f"w_sb{h % 2}")
                nc.sync.dma_start(
                    out=w_sb,
                    in_=w[h * 128 : (h + 1) * 128, :]
                    .rearrange("p (n m) -> p n m", m=128),
                )
                w_sbs.append(w_sb)
            return w_sbs

        def make_qk(h, w_sbs):
            """Generator: emits QK_h matmuls a few at a time; returns tiles
            immediately (they are filled as the generator is drained)."""
            qt_t = qkp.tile([128, S], BF16, tag="qt", name=f"qt{h % 2}")
            kt_t = qkp.tile([128, S], BF16, tag="kt", name=f"kt{h % 2}")

            def gen():
                for w_sb, b_sb, dst in (
                    (w_sbs[0], bq_sb, qt_t),
                    (w_sbs[1], bk_sb, kt_t),
                ):
                    for sq in range(4):
                        psum = ps8.tile([128, 512], F32, tag="qk", name="qk_ps")
                        for kd in range(KT):
                            nc.tensor.matmul(
                                psum,
                                w_sb[:, kd, :],
                                xt[:, sq, :, kd, :],
                                start=(kd == 0),
                                stop=(kd == KT - 1),
                            )
                            yield
                        nc.vector.tensor_scalar_add(
                            out=dst[:, sq * 512 : (sq + 1) * 512],
                            in0=psum,
                            scalar1=b_sb[:, h : h + 1],
                        )
                while True:
                    yield

            return qt_t, kt_t, gen()

        # ------------- attention per head, QK_{h+1} interleaved ----------
        def wo_load(ncol):
            wo_c = wbig.tile([128, HPC, 512], BF16, tag="wbig",
                             name=f"wo_c{ncol}")
            nc.sync.dma_start(out=wo_c, in_=wo_r[:, ncol])
            return wo_c

        wo_chunks = []
        w_pre = {0: prep_w(0), 1: prep_w(1)}
        qt_cur, kt_cur, g0 = make_qk(0, w_pre.pop(0))
        for _ in range(140):
            next(g0)

        for h in range(HPC):
            if h + 2 < HPC:
                w_pre[h + 2] = prep_w(h + 2)
            if h + 1 < HPC:
                qt_nxt, kt_nxt, gnxt = make_qk(h + 1, w_pre.pop(h + 1))
            else:
                qt_nxt = kt_nxt = gnxt = None

            def fill(n):
                if gnxt is not None:
                    for _ in range(n):
                        next(gnxt)

            if h == HPC - 1:
                # last head: prefetch all Wo chunks for phase C instead
                for ncol in range(4):
                    wo_chunks.append(wo_load(ncol))

            for qc in range(QC):
                nkt = 4 * qc + 4
                # diagonal tiles first: their longer PE->ACT->DVE chains
                # start early and overlap with the full tiles' stream
                order = list(range(4 * qc, nkt)) + list(range(4 * qc))
                psum_c = ps8.tile([128, 512], F32, tag="c", name="psum_c")
                psum_s = ps8.tile([128, 512], F32, tag="s", name="psum_s")

                def scores(kt_i):
                    # diagonal tile j has valid columns only at qq >= 128j:
                    # compute just that [128, 512-128j] strip
                    j = kt_i - 4 * qc
                    off = 128 * j if j > 0 else 0
                    ps_t = ps8.tile([128, 512], F32, tag="st", name="ps_t")
                    nc.tensor.matmul(
                        ps_t[:, off:],
                        kt_cur[:, kt_i * 128 : (kt_i + 1) * 128],
                        qt_cur[:, qc * 512 + off : (qc + 1) * 512],
                        start=True,
                        stop=True,
                    )
                    p_t = ppool.tile([128, 512], BF16, tag="p_t")
                    nc.scalar.activation(
                        out=p_t[:, off:],
                        in_=ps_t[:, off:],
                        func=mybir.ActivationFunctionType.Exp,
                        scale=float(SCALE),
                    )
                    if j >= 0:
                        # zero p where k > q: only possible in the first
                        # 128 columns of the strip
                        nc.vector.tensor_tensor(
                            out=p_t[:, off : off + 128],
                            in0=p_t[:, off : off + 128],
                            in1=mb_sb,
                            op=fmul,
                        )
                    return p_t, off

                def ctx(idx, p_t, off, kt_i):
                    nc.tensor.matmul(
                        psum_c[:, off:],
                        v_sb[:, kt_i, h * 128 : (h + 1) * 128],
                        p_t[:, off:],
                        start=(idx == 0),
                        stop=(idx == nkt - 1),
                    )
                    nc.tensor.matmul(
                        psum_s[:, off:],
                        ones_bf,
                        p_t[:, off:],
                        start=(idx == 0),
                        stop=(idx == nkt - 1),
                    )

                # software-pipeline scores/exp ahead of ctx by one tile;
                # pad the PE queue with next head's projection matmuls
                prev = None
                for idx, kt_i in enumerate(order):
                    p_t, off = scores(kt_i)
                    fill(3)
                    if prev is not None:
                        ctx(idx - 1, prev[0], prev[1], prev[2])
                    prev = (p_t, off, kt_i)
                ctx(nkt - 1, prev[0], prev[1], prev[2])
                fill(2)

                recip = rcpool.tile([128, 512], F32, tag="rcp")
                nc.vector.reciprocal_approx_fast(out=recip, in_=psum_s)
                nc.vector.tensor_tensor(
                    out=ct[:, h, qc * 512 : (qc + 1) * 512],
                    in0=psum_c,
                    in1=recip,
                    op=fmul,
                )
            fill(200)  # drain any remainder of QK_{h+1}
            qt_cur, kt_cur = qt_nxt, kt_nxt

        # ---------------- output projection -------------------------
        # st-outer so each s-row-block [128, 2048] stores as one DMA of
        # fully contiguous 4KB rows
        out_r = out.rearrange("s (c m) -> s c m", m=512)
        for st in range(ST):
            o_sb = ost.tile([128, 4, 512], BF16, tag="ostage")
            for ncol in range(4):
                psum = ps8.tile([128, 512], F32,
                                tag=("qk", "st", "c", "s")[ncol], name="o_ps")
                for hh in range(HPC):
                    nc.tensor.matmul(
                        psum,
                        ct[:, hh, st * 128 : (st + 1) * 128],
                        wo_chunks[ncol][:, hh, :],
                        start=(hh == 0),
                        stop=(hh == HPC - 1),
                    )
                nc.scalar.activation(
                    out=o_sb[:, ncol, :],
                    in_=psum,
                    func=mybir.ActivationFunctionType.Copy,
                )
            nc.gpsimd.dma_start(
                out=out_r[st * 128 : (st + 1) * 128], in_=o_sb
            )


_NC = None


def _get_nc():
    global _NC
    if _NC is None:
        _NC = _build_nc()
    return _NC


def _host_prep(input_sequences, Wq, bq, Wk, bk, Wv, bv, Wo, bo):
    """Build per-core input maps."""
    x = np.asarray(input_sequences, dtype=np.float32)
    mbm = (np.arange(128)[:, None] <= np.arange(128)[None, :]).astype(BF16_NP)

    in_maps = []
    for c in range(8):
        b, g = divmod(c, 2)
        sl = slice(g * DHG, (g + 1) * DHG)
        wq_c = np.ascontiguousarray(
            np.asarray(Wq[:, sl], dtype=np.float32)
            .reshape(KT, 128, HPC, 128).transpose(2, 1, 0, 3).reshape(DHG, D)
        ).astype(BF16_NP)
        wk_c = np.ascontiguousarray(
            np.asarray(Wk[:, sl], dtype=np.float32)
            .reshape(KT, 128, HPC, 128).transpose(2, 1, 0, 3).reshape(DHG, D)
        ).astype(BF16_NP)
        # wv packed [t2*128+p, kd*512+m] = Wv[kd*128+p, t2*512+m]
        wv_c = np.ascontiguousarray(
            np.asarray(Wv[:, sl], dtype=np.float32)
            .reshape(KT, 128, 2, 512).transpose(2, 1, 0, 3).reshape(256, KT * 512)
        ).astype(BF16_NP)
        # wo packed [p, ncol*4096 + h*512 + m] = Wo[sl][h*128+p, ncol*512+m]
        wo_c = np.ascontiguousarray(
            np.asarray(Wo[sl, :], dtype=np.float32)
            .reshape(HPC, 128, 4, 512).transpose(1, 2, 0, 3).reshape(128, HPC * D)
        ).astype(BF16_NP)
        # xT packed [sq*128+p, blk*2048 + kd*128 + u]
        #   = x[sq*512 + blk*128 + u, kd*128+p]
        xt_c = np.ascontiguousarray(
            x[b].reshape(4, 4, 128, KT, 128).transpose(0, 4, 1, 3, 2)
            .reshape(512, KT * 512)
        ).astype(BF16_NP)
        in_maps.append({
            "xT": xt_c,
            "wq": wq_c,
            "wk": wk_c,
            "wv": wv_c,
            "wo": wo_c,
            "bqT": np.ascontiguousarray(
                np.asarray(bq[sl], dtype=np.float32).reshape(HPC, 128).T
            ),
            "bkT": np.ascontiguousarray(
                np.asarray(bk[sl], dtype=np.float32).reshape(HPC, 128).T
            ),
            "bvb": np.ascontiguousarray(
                np.broadcast_to(np.asarray(bv[sl], dtype=np.float32), (128, DHG))
            ),
            "mb": mbm,
        })
    return in_maps


def kernel(input_sequences, Wq, bq, Wk, bk, Wv, bv, Wo, bo, _trace=False):
    nc = _get_nc()
    in_maps = _host_prep(input_sequences, Wq, bq, Wk, bk, Wv, bv, Wo, bo)
    res = run_bass_kernel_spmd(nc, in_maps, list(range(8)), trace=_trace)
    bo32 = np.asarray(bo, dtype=np.float32)
    out = np.empty((B, S, D), dtype=np.float32)
    for b in range(B):
        out[b] = (
            res.results[2 * b]["out"].astype(np.float32)
            + res.results[2 * b + 1]["out"].astype(np.float32)
            + bo32
        )
    if _trace:
        kernel.last_exec_time_ns = res.exec_time_ns
    return out
